# revision 5
# baseline (speedup 1.0000x reference)
"""Trainium2 Bass kernel for a dense MHA layer (B=2, S=2048, H=1024, 16 heads)
with residual + LayerNorm, tensor-parallel over heads across 8 NeuronCores.

Per-core plan (core c owns heads 2c, 2c+1):
  phase 1: QKV projections from a shared transposed activation (xT), keeping
           Q^T/K^T feature-major and V token-major (with a ones column so the
           attention matmul also produces softmax denominators).
  phase 2: per (batch, head, q-stripe): scores^T = K Q^T on PE, exp via ACT
           (mask folded into the per-partition bias, 1/sqrt(hd) into the
           scale), then att^T = [V|1]^T E accumulated over k-tiles.
  phase 3: AllToAll re-shards from head-parallel to sequence-parallel
           (each core ends with all 1024 att features for its 512 tokens,
           plus the 16 per-head denominators as extra rows).
  phase 4: normalize, output projection, residual add, LayerNorm.

All matmuls run in bf16 with fp32 PSUM accumulation; softmax denominators and
the LayerNorm path stay fp32.
"""

import sys

for _p in ("/opt/trn_rl_repo", "/root/.axon_site/_ro/trn_rl_repo"):
    if _p not in sys.path:
        sys.path.append(_p)

import numpy as np
import ml_dtypes

import concourse.bacc as bacc
import concourse.tile as tile
import concourse.mybir as mybir
from concourse.bass_utils import run_bass_kernel_spmd

F32 = mybir.dt.float32
BF16 = mybir.dt.bfloat16
AF = mybir.ActivationFunctionType
ALU = mybir.AluOpType

NC = 8          # cores
H = 1024        # model dim
NH = 16         # heads
HD = 64         # head dim
B = 2
S = 2048
T = B * S       # 4096 tokens
TPC = T // NC   # 512 tokens per core (phase 4)
NSTR = T // 512  # 8 token stripes of 512
KT = S // 128   # 16 k-tiles per batch
QS = S // 512   # 4 q-stripes per batch
EPS = 1e-12

_RUNNER = None


def _build_program():
    nc = bacc.Bacc("TRN2", target_bir_lowering=False, debug=False, num_devices=NC)

    xT = nc.dram_tensor("xT", [H, T], BF16, kind="ExternalInput")
    wq = nc.dram_tensor("wq", [H, 128], BF16, kind="ExternalInput")
    wk = nc.dram_tensor("wk", [H, 128], BF16, kind="ExternalInput")
    wv = nc.dram_tensor("wv", [H, 128], BF16, kind="ExternalInput")
    bq = nc.dram_tensor("bq", [128, 1], F32, kind="ExternalInput")
    bk = nc.dram_tensor("bk", [128, 1], F32, kind="ExternalInput")
    bv = nc.dram_tensor("bv", [1, 128], BF16, kind="ExternalInput")
    mneg = nc.dram_tensor("mneg", [128, B * KT], F32, kind="ExternalInput")
    wot = nc.dram_tensor("wot", [H, H], BF16, kind="ExternalInput")
    resi = nc.dram_tensor("resi", [TPC, H], F32, kind="ExternalInput")
    lnw = nc.dram_tensor("lnw", [128, H], F32, kind="ExternalInput")
    lnb = nc.dram_tensor("lnb", [128, H], F32, kind="ExternalInput")
    y = nc.dram_tensor("y", [TPC, H], F32, kind="ExternalOutput")

    with tile.TileContext(nc) as tc:
        with (
            tc.tile_pool(name="const", bufs=1) as constp,
            tc.tile_pool(name="pers", bufs=1) as pers,
            tc.tile_pool(name="work", bufs=2) as workp,
            tc.tile_pool(name="ps", bufs=1, space="PSUM") as ps,
            tc.tile_pool(name="dram", bufs=1, space="DRAM") as dram,
        ):
            # ---- constants / weights
            wq_sb = constp.tile([128, 8, 128], BF16)
            nc.sync.dma_start(wq_sb[:], wq.ap().rearrange("(k p) m -> p k m", p=128))
            wk_sb = constp.tile([128, 8, 128], BF16)
            nc.sync.dma_start(wk_sb[:], wk.ap().rearrange("(k p) m -> p k m", p=128))
            wv_sb = constp.tile([128, 8, 128], BF16)
            nc.sync.dma_start(wv_sb[:], wv.ap().rearrange("(k p) m -> p k m", p=128))
            wot_sb = constp.tile([128, 8, H], BF16)
            nc.sync.dma_start(wot_sb[:], wot.ap().rearrange("(j p) f -> p j f", p=128))
            bq_sb = constp.tile([128, 1], F32)
            nc.sync.dma_start(bq_sb[:], bq.ap())
            bk_sb = constp.tile([128, 1], F32)
            nc.sync.dma_start(bk_sb[:], bk.ap())
            bv_sb = constp.tile([1, 128], BF16)
            nc.sync.dma_start(bv_sb[:], bv.ap())
            mneg_sb = constp.tile([128, B * KT], F32)
            nc.sync.dma_start(mneg_sb[:], mneg.ap())
            lnw_sb = constp.tile([128, H], F32)
            nc.sync.dma_start(lnw_sb[:], lnw.ap())
            lnb_sb = constp.tile([128, H], F32)
            nc.sync.dma_start(lnb_sb[:], lnb.ap())
            ones_sb = constp.tile([1, 128], BF16)
            nc.vector.memset(ones_sb[:], 1.0)
            eps_sb = constp.tile([128, 1], F32)
            nc.vector.memset(eps_sb[:], EPS)

            qt_sb = pers.tile([128, T], BF16)   # Q^T (2 heads stacked)
            kt_sb = pers.tile([128, T], BF16)   # K^T
            v_sb = pers.tile([128, 32, 130], BF16)  # V tok-major + ones cols
            att_sb = pers.tile([128, 8, 512], BF16)  # normalized att^T (phase 4)
            nc.vector.memset(v_sb[:, :, 64:65], 1.0)
            nc.vector.memset(v_sb[:, :, 129:130], 1.0)

            a2a_in = dram.tile([NC, 130, 512], F32)
            a2a_out = dram.tile([NC, 130, 512], F32)

            xTr = xT.ap().rearrange("(k p) (s t) -> s p k t", p=128, t=512)

            # ---- phase 1: QKV projections
            for s in range(NSTR):
                xs = workp.tile([128, 8, 512], BF16, tag="xs", bufs=2)
                nc.sync.dma_start(xs[:], xTr[s])

                qp = ps.tile([128, 512], F32, tag="mm1", bufs=4)
                for k in range(8):
                    nc.tensor.matmul(
                        qp[:], wq_sb[:, k, :], xs[:, k, :], start=(k == 0), stop=(k == 7)
                    )
                nc.scalar.activation(
                    qt_sb[:, 512 * s : 512 * (s + 1)], qp[:], AF.Identity, bias=bq_sb[:]
                )

                kp = ps.tile([128, 512], F32, tag="mm1", bufs=4)
                for k in range(8):
                    nc.tensor.matmul(
                        kp[:], wk_sb[:, k, :], xs[:, k, :], start=(k == 0), stop=(k == 7)
                    )
                nc.scalar.activation(
                    kt_sb[:, 512 * s : 512 * (s + 1)], kp[:], AF.Identity, bias=bk_sb[:]
                )

                for tt in range(4):
                    vp = ps.tile([128, 128], F32, tag="acc", bufs=2)
                    for k in range(8):
                        nc.tensor.matmul(
                            vp[:],
                            xs[:, k, 128 * tt : 128 * (tt + 1)],
                            wv_sb[:, k, :],
                            start=(k == 0),
                            stop=False,
                        )
                    nc.tensor.matmul(vp[:], ones_sb[:], bv_sb[:], start=False, stop=True)
                    g = s * 4 + tt
                    nc.vector.tensor_copy(v_sb[:, g, 0:64], vp[:, 0:64])
                    nc.vector.tensor_copy(v_sb[:, g, 65:129], vp[:, 64:128])

            # ---- phase 2: attention (per batch, local head, q-stripe)
            for b in range(B):
                for lh in range(2):
                    hr = 64 * lh
                    for qs in range(QS):
                        j = b * QS + qs
                        qcol = b * S + 512 * qs
                        e_sb = workp.tile([128, KT, 512], BF16, tag="e", bufs=2)
                        for kt in range(KT):
                            kcol = b * S + 128 * kt
                            sp = ps.tile([128, 512], F32, tag="mm1", bufs=4)
                            nc.tensor.matmul(
                                sp[:],
                                kt_sb[hr : hr + 64, kcol : kcol + 128],
                                qt_sb[hr : hr + 64, qcol : qcol + 512],
                                start=True,
                                stop=True,
                            )
                            nc.scalar.activation(
                                e_sb[:, kt, :],
                                sp[:],
                                AF.Exp,
                                bias=mneg_sb[:, b * KT + kt : b * KT + kt + 1],
                                scale=1.0 / np.sqrt(HD),
                            )
                        av = ps.tile([65, 512], F32, tag="acc", bufs=2)
                        for kt in range(KT):
                            g = b * KT + kt
                            nc.tensor.matmul(
                                av[:],
                                v_sb[:, g, 65 * lh : 65 * lh + 65],
                                e_sb[:, kt, :],
                                start=(kt == 0),
                                stop=(kt == KT - 1),
                            )
                        avs = workp.tile([65, 512], F32, tag="avs", bufs=2)
                        nc.vector.tensor_copy(avs[:], av[:])
                        nc.sync.dma_start(a2a_in[j, 64 * lh : 64 * lh + 64, :], avs[0:64, :])
                        nc.sync.dma_start(a2a_in[j, 128 + lh : 129 + lh, :], avs[64:65, :])

            # ---- phase 3: AllToAll (head-parallel -> sequence-parallel)
            nc.gpsimd.collective_compute(
                "AllToAll",
                ALU.bypass,
                replica_groups=[list(range(NC))],
                ins=[a2a_in.opt()],
                outs=[a2a_out.opt()],
            )

            # ---- phase 4: normalize + output projection + residual + LayerNorm
            sums_sb = workp.tile([16, 512], F32, tag="sums", bufs=1)
            nc.sync.dma_start(sums_sb[:], a2a_out[0:NC, 128:130, :])
            recip_sb = workp.tile([16, 512], F32, tag="recip", bufs=1)
            nc.vector.reciprocal(recip_sb[:], sums_sb[:])

            for j in range(NC):
                blk = workp.tile([128, 512], F32, tag="blk", bufs=2)
                nc.sync.dma_start(blk[:], a2a_out[j, 0:128, :])
                rb = workp.tile([128, 512], F32, tag="rb", bufs=2)
                nc.sync.dma_start(
                    rb[:],
                    recip_sb[2 * j : 2 * j + 2, :].unsqueeze(1).broadcast_to([2, 64, 512]),
                )
                nc.vector.tensor_tensor(att_sb[:, j, :], blk[:], rb[:], ALU.mult)

            for tt in range(4):
                x_sb = workp.tile([128, H], F32, tag="xsb", bufs=2)
                for ft in range(2):
                    op = ps.tile([128, 512], F32, tag="mm1", bufs=4)
                    for j in range(NC):
                        nc.tensor.matmul(
                            op[:],
                            att_sb[:, j, 128 * tt : 128 * (tt + 1)],
                            wot_sb[:, j, 512 * ft : 512 * (ft + 1)],
                            start=(j == 0),
                            stop=(j == NC - 1),
                        )
                    res_t = workp.tile([128, 512], F32, tag="res", bufs=2)
                    nc.sync.dma_start(
                        res_t[:],
                        resi.ap()[128 * tt : 128 * (tt + 1), 512 * ft : 512 * (ft + 1)],
                    )
                    nc.vector.tensor_tensor(
                        x_sb[:, 512 * ft : 512 * (ft + 1)], op[:], res_t[:], ALU.add
                    )

                bnst = workp.tile([128, 2, 6], F32, tag="bnst", bufs=2)
                nc.vector.bn_stats(bnst[:, 0, :], x_sb[:, 0:512])
                nc.vector.bn_stats(bnst[:, 1, :], x_sb[:, 512:1024])
                stats = workp.tile([128, 2], F32, tag="stats", bufs=2)
                nc.vector.bn_aggr(stats[:], bnst[:])
                std = workp.tile([128, 1], F32, tag="std", bufs=2)
                nc.scalar.activation(std[:], stats[:, 1:2], AF.Sqrt, bias=eps_sb[:])
                rstd = workp.tile([128, 1], F32, tag="rstd", bufs=2)
                nc.vector.reciprocal(rstd[:], std[:])
                nmr = workp.tile([128, 1], F32, tag="nmr", bufs=2)
                nc.vector.tensor_scalar(
                    nmr[:], stats[:, 0:1], rstd[:], -1.0, ALU.mult, ALU.mult
                )
                xh = workp.tile([128, H], F32, tag="xh", bufs=2)
                nc.vector.tensor_scalar(
                    xh[:], x_sb[:], rstd[:], nmr[:], ALU.mult, ALU.add
                )
                yt = workp.tile([128, H], F32, tag="yt", bufs=2)
                nc.vector.tensor_tensor(yt[:], xh[:], lnw_sb[:], ALU.mult)
                nc.vector.tensor_tensor(yt[:], yt[:], lnb_sb[:], ALU.add)
                nc.sync.dma_start(y.ap()[128 * tt : 128 * (tt + 1), :], yt[:])

    nc.compile()
    return nc


class _Runner:
    """Compiles the Bass program once and keeps a reusable sharded jit."""

    def __init__(self):
        self.nc = _build_program()
        self._sharded = None
        self._meta = None

    def _make_sharded(self):
        import jax
        from jax.sharding import Mesh, PartitionSpec
        from jax.experimental.shard_map import shard_map
        from concourse.bass2jax import (
            _bass_exec_p,
            install_neuronx_cc_hook,
            partition_id_tensor,
        )

        install_neuronx_cc_hook()
        nc = self.nc
        partition_name = (
            nc.partition_id_tensor.name if nc.partition_id_tensor else None
        )

        in_names, out_names, out_avals, zero_outs = [], [], [], []
        for alloc in nc.m.functions[0].allocations:
            if not isinstance(alloc, mybir.MemoryLocationSet):
                continue
            name = alloc.memorylocations[0].name
            if alloc.kind == "ExternalInput":
                if name != partition_name:
                    in_names.append(name)
            elif alloc.kind == "ExternalOutput":
                shape = tuple(alloc.tensor_shape)
                dtype = mybir.dt.np(alloc.dtype)
                out_names.append(name)
                out_avals.append(jax.core.ShapedArray(shape, dtype))
                zero_outs.append(np.zeros(shape, dtype))
        n_params = len(in_names)
        all_names = list(in_names) + list(out_names)
        if partition_name is not None:
            all_names.append(partition_name)

        def _body(*args):
            operands = list(args)
            if partition_name is not None:
                operands.append(partition_id_tensor())
            outs = _bass_exec_p.bind(
                *operands,
                out_avals=tuple(out_avals),
                in_names=tuple(all_names),
                out_names=tuple(out_names),
                lowering_input_output_aliases=(),
                sim_require_finite=True,
                sim_require_nnan=True,
                nc=nc,
            )
            return tuple(outs)

        devices = jax.devices()[:NC]
        mesh = Mesh(np.asarray(devices), ("core",))
        n_outs = len(out_names)
        in_specs = (PartitionSpec("core"),) * (n_params + n_outs)
        out_specs = (PartitionSpec("core"),) * n_outs
        donate = tuple(range(n_params, n_params + n_outs))
        sharded = jax.jit(
            shard_map(
                _body, mesh=mesh, in_specs=in_specs, out_specs=out_specs, check_rep=False
            ),
            donate_argnums=donate,
            keep_unused=True,
        )
        self._meta = (in_names, out_names, out_avals, zero_outs)
        self._sharded = sharded

    def run(self, in_maps):
        if self._sharded is None:
            self._make_sharded()
        in_names, out_names, out_avals, zero_outs = self._meta
        n_params = len(in_names)
        concat_in = [
            np.concatenate([np.asarray(m[name]) for m in in_maps], axis=0)
            for name in in_names
        ]
        concat_zeros = [
            np.zeros((NC * z.shape[0], *z.shape[1:]), z.dtype) for z in zero_outs
        ]
        out_arrs = self._sharded(*concat_in, *concat_zeros)
        return [
            {
                name: np.asarray(out_arrs[i]).reshape(NC, *out_avals[i].shape)[c]
                for i, name in enumerate(out_names)
            }
            for c in range(NC)
        ]


def _get_runner():
    global _RUNNER
    if _RUNNER is None:
        _RUNNER = _Runner()
    return _RUNNER


def _prep_in_maps(pre_out, att_mask, Wq, bq, Wk, bk, Wv, bv, Wo, bo, ln_w, ln_b):
    f32 = np.float32
    bf16 = ml_dtypes.bfloat16
    x = np.asarray(pre_out, f32).reshape(T, H)
    xT = np.ascontiguousarray(x.T).astype(bf16)

    m = (1.0 - np.asarray(att_mask, f32).reshape(B, S)) * -10000.0
    # column (b*KT + kt) holds mask for k-tokens [kt*128, (kt+1)*128) of batch b
    mneg = np.ascontiguousarray(m.reshape(B, KT, 128).transpose(2, 0, 1).reshape(128, B * KT))

    wot = np.ascontiguousarray(np.asarray(Wo, f32).T).astype(bf16)
    res_full = x + np.asarray(bo, f32)[None, :]
    lnw_b = np.ascontiguousarray(np.broadcast_to(np.asarray(ln_w, f32), (128, H)))
    lnb_b = np.ascontiguousarray(np.broadcast_to(np.asarray(ln_b, f32), (128, H)))

    Wq_, Wk_, Wv_ = (np.asarray(w, f32) for w in (Wq, Wk, Wv))
    bq_, bk_, bv_ = (np.asarray(v, f32) for v in (bq, bk, bv))

    in_maps = []
    for c in range(NC):
        fs = slice(128 * c, 128 * (c + 1))
        in_maps.append(
            {
                "xT": xT,
                "wq": np.ascontiguousarray(Wq_[fs].T).astype(bf16),
                "wk": np.ascontiguousarray(Wk_[fs].T).astype(bf16),
                "wv": np.ascontiguousarray(Wv_[fs].T).astype(bf16),
                "bq": np.ascontiguousarray(bq_[fs].reshape(128, 1)),
                "bk": np.ascontiguousarray(bk_[fs].reshape(128, 1)),
                "bv": np.ascontiguousarray(bv_[fs].reshape(1, 128)).astype(bf16),
                "mneg": mneg,
                "wot": wot,
                "resi": np.ascontiguousarray(res_full[TPC * c : TPC * (c + 1)]),
                "lnw": lnw_b,
                "lnb": lnb_b,
            }
        )
    return in_maps


def kernel(**inputs):
    runner = _get_runner()
    in_maps = _prep_in_maps(**inputs)
    results = runner.run(in_maps)
    y = np.concatenate([results[c]["y"] for c in range(NC)], axis=0)
    return y.reshape(B, S, H).astype(np.float32)


# revision 8
# speedup vs baseline: 1218.4121x; 1218.4121x over previous
"""Trainium2 Bass kernel for a dense MHA layer (B=2, S=2048, H=1024, 16 heads)
with residual + LayerNorm, tensor-parallel over heads across 8 NeuronCores.

Per-core plan (core c owns heads 2c, 2c+1):
  phase 1: QKV projections from a shared transposed activation (xT), keeping
           Q^T/K^T feature-major and V token-major (with a ones column so the
           attention matmul also produces softmax denominators).
  phase 2: per (batch, head, q-stripe): scores^T = K Q^T on PE, exp via ACT
           (mask folded into the per-partition bias, 1/sqrt(hd) into the
           scale), then att^T = [V|1]^T E accumulated over k-tiles.
  phase 3: AllToAll re-shards from head-parallel to sequence-parallel
           (each core ends with all 1024 att features for its 512 tokens,
           plus the 16 per-head denominators as extra rows).
  phase 4: normalize, output projection, residual add, LayerNorm.

All matmuls run in bf16 with fp32 PSUM accumulation; softmax denominators and
the LayerNorm path stay fp32.
"""

import sys

for _p in ("/opt/trn_rl_repo", "/root/.axon_site/_ro/trn_rl_repo"):
    if _p not in sys.path:
        sys.path.append(_p)

import numpy as np
import ml_dtypes

import concourse.bacc as bacc
import concourse.tile as tile
import concourse.mybir as mybir
from concourse.bass_utils import run_bass_kernel_spmd

F32 = mybir.dt.float32
BF16 = mybir.dt.bfloat16
AF = mybir.ActivationFunctionType
ALU = mybir.AluOpType

NC = 8          # cores
H = 1024        # model dim
NH = 16         # heads
HD = 64         # head dim
B = 2
S = 2048
T = B * S       # 4096 tokens
TPC = T // NC   # 512 tokens per core (phase 4)
NSTR = T // 512  # 8 token stripes of 512
KT = S // 128   # 16 k-tiles per batch
QS = S // 512   # 4 q-stripes per batch
EPS = 1e-12

_RUNNER = None


def _build_program():
    nc = bacc.Bacc("TRN2", target_bir_lowering=False, debug=False, num_devices=NC)

    xT = nc.dram_tensor("xT", [H, T], BF16, kind="ExternalInput")
    wq = nc.dram_tensor("wq", [H, 128], BF16, kind="ExternalInput")
    wk = nc.dram_tensor("wk", [H, 128], BF16, kind="ExternalInput")
    wv = nc.dram_tensor("wv", [H, 128], BF16, kind="ExternalInput")
    bq = nc.dram_tensor("bq", [128, 1], F32, kind="ExternalInput")
    bk = nc.dram_tensor("bk", [128, 1], F32, kind="ExternalInput")
    bv = nc.dram_tensor("bv", [1, 128], BF16, kind="ExternalInput")
    mneg = nc.dram_tensor("mneg", [128, B * KT], F32, kind="ExternalInput")
    wot = nc.dram_tensor("wot", [H, H], BF16, kind="ExternalInput")
    resi = nc.dram_tensor("resi", [TPC, H], F32, kind="ExternalInput")
    lnw = nc.dram_tensor("lnw", [128, H], F32, kind="ExternalInput")
    lnb = nc.dram_tensor("lnb", [128, H], F32, kind="ExternalInput")
    y = nc.dram_tensor("y", [TPC, H], F32, kind="ExternalOutput")

    with tile.TileContext(nc) as tc:
        with (
            tc.tile_pool(name="const", bufs=1) as constp,
            tc.tile_pool(name="pers", bufs=1) as pers,
            tc.tile_pool(name="work", bufs=2) as workp,
            tc.tile_pool(name="ps", bufs=1, space="PSUM") as ps,
            tc.tile_pool(name="dram", bufs=1, space="DRAM") as dram,
        ):
            # ---- constants / weights
            wq_sb = constp.tile([128, 8, 128], BF16)
            nc.sync.dma_start(wq_sb[:], wq.ap().rearrange("(k p) m -> p k m", p=128))
            wk_sb = constp.tile([128, 8, 128], BF16)
            nc.sync.dma_start(wk_sb[:], wk.ap().rearrange("(k p) m -> p k m", p=128))
            wv_sb = constp.tile([128, 8, 128], BF16)
            nc.sync.dma_start(wv_sb[:], wv.ap().rearrange("(k p) m -> p k m", p=128))
            wot_sb = constp.tile([128, 8, H], BF16)
            nc.sync.dma_start(wot_sb[:], wot.ap().rearrange("(j p) f -> p j f", p=128))
            bq_sb = constp.tile([128, 1], F32)
            nc.sync.dma_start(bq_sb[:], bq.ap())
            bk_sb = constp.tile([128, 1], F32)
            nc.sync.dma_start(bk_sb[:], bk.ap())
            bv_sb = constp.tile([1, 128], BF16)
            nc.sync.dma_start(bv_sb[:], bv.ap())
            mneg_sb = constp.tile([128, B * KT], F32)
            nc.sync.dma_start(mneg_sb[:], mneg.ap())
            lnw_sb = constp.tile([128, H], F32)
            nc.sync.dma_start(lnw_sb[:], lnw.ap())
            lnb_sb = constp.tile([128, H], F32)
            nc.sync.dma_start(lnb_sb[:], lnb.ap())
            ones_sb = constp.tile([1, 128], BF16)
            nc.vector.memset(ones_sb[:], 1.0)
            eps_sb = constp.tile([128, 1], F32)
            nc.vector.memset(eps_sb[:], EPS)

            qt_sb = pers.tile([128, T], BF16)   # Q^T (2 heads stacked)
            kt_sb = pers.tile([128, T], BF16)   # K^T
            v_sb = pers.tile([128, 32, 130], BF16)  # V tok-major + ones cols
            att_sb = pers.tile([128, 8, 512], BF16)  # normalized att^T (phase 4)
            nc.vector.memset(v_sb[:, :, 64:65], 1.0)
            nc.vector.memset(v_sb[:, :, 129:130], 1.0)

            a2a_in = dram.tile([NC, 130, 512], F32)
            a2a_out = dram.tile([NC, 130, 512], F32)

            xTr = xT.ap().rearrange("(k p) (s t) -> s p k t", p=128, t=512)

            # ---- phase 1: QKV projections
            for s in range(NSTR):
                xs = workp.tile([128, 8, 512], BF16, tag="xs", bufs=2)
                nc.sync.dma_start(xs[:], xTr[s])

                qp = ps.tile([128, 512], F32, tag="mm1", bufs=4)
                for k in range(8):
                    nc.tensor.matmul(
                        qp[:], wq_sb[:, k, :], xs[:, k, :], start=(k == 0), stop=(k == 7)
                    )
                nc.scalar.activation(
                    qt_sb[:, 512 * s : 512 * (s + 1)], qp[:], AF.Identity, bias=bq_sb[:]
                )

                kp = ps.tile([128, 512], F32, tag="mm1", bufs=4)
                for k in range(8):
                    nc.tensor.matmul(
                        kp[:], wk_sb[:, k, :], xs[:, k, :], start=(k == 0), stop=(k == 7)
                    )
                nc.scalar.activation(
                    kt_sb[:, 512 * s : 512 * (s + 1)], kp[:], AF.Identity, bias=bk_sb[:]
                )

                for tt in range(4):
                    vp = ps.tile([128, 128], F32, tag="acc", bufs=2)
                    for k in range(8):
                        nc.tensor.matmul(
                            vp[:],
                            xs[:, k, 128 * tt : 128 * (tt + 1)],
                            wv_sb[:, k, :],
                            start=(k == 0),
                            stop=False,
                        )
                    nc.tensor.matmul(vp[:], ones_sb[:], bv_sb[:], start=False, stop=True)
                    g = s * 4 + tt
                    nc.vector.tensor_copy(v_sb[:, g, 0:64], vp[:, 0:64])
                    nc.vector.tensor_copy(v_sb[:, g, 65:129], vp[:, 64:128])

            # ---- phase 2: attention (per batch, local head, q-stripe)
            for b in range(B):
                for lh in range(2):
                    hr = 64 * lh
                    for qs in range(QS):
                        j = b * QS + qs
                        qcol = b * S + 512 * qs
                        e_sb = workp.tile([128, KT, 512], BF16, tag="e", bufs=2)
                        for kt in range(KT):
                            kcol = b * S + 128 * kt
                            sp = ps.tile([128, 512], F32, tag="mm1", bufs=4)
                            nc.tensor.matmul(
                                sp[:],
                                kt_sb[hr : hr + 64, kcol : kcol + 128],
                                qt_sb[hr : hr + 64, qcol : qcol + 512],
                                start=True,
                                stop=True,
                            )
                            nc.scalar.activation(
                                e_sb[:, kt, :],
                                sp[:],
                                AF.Exp,
                                bias=mneg_sb[:, b * KT + kt : b * KT + kt + 1],
                                scale=1.0 / np.sqrt(HD),
                            )
                        av = ps.tile([65, 512], F32, tag="acc", bufs=2)
                        for kt in range(KT):
                            g = b * KT + kt
                            nc.tensor.matmul(
                                av[:],
                                v_sb[:, g, 65 * lh : 65 * lh + 65],
                                e_sb[:, kt, :],
                                start=(kt == 0),
                                stop=(kt == KT - 1),
                            )
                        avs = workp.tile([65, 512], F32, tag="avs", bufs=2)
                        nc.vector.tensor_copy(avs[:], av[:])
                        nc.sync.dma_start(a2a_in[j, 64 * lh : 64 * lh + 64, :], avs[0:64, :])
                        nc.sync.dma_start(a2a_in[j, 128 + lh : 129 + lh, :], avs[64:65, :])

            # ---- phase 3: AllToAll (head-parallel -> sequence-parallel)
            nc.gpsimd.collective_compute(
                "AllToAll",
                ALU.bypass,
                replica_groups=[list(range(NC))],
                ins=[a2a_in.opt()],
                outs=[a2a_out.opt()],
            )

            # ---- phase 4: normalize + output projection + residual + LayerNorm
            sums_sb = workp.tile([16, 512], F32, tag="sums", bufs=1)
            nc.sync.dma_start(sums_sb[:], a2a_out[0:NC, 128:130, :])
            recip_sb = workp.tile([16, 512], F32, tag="recip", bufs=1)
            nc.vector.reciprocal(recip_sb[:], sums_sb[:])

            for j in range(NC):
                blk = workp.tile([128, 512], F32, tag="blk", bufs=2)
                nc.sync.dma_start(blk[:], a2a_out[j, 0:128, :])
                rb = workp.tile([128, 512], F32, tag="rb", bufs=2)
                nc.sync.dma_start(
                    rb[:],
                    recip_sb[2 * j : 2 * j + 2, :].unsqueeze(1).broadcast_to([2, 64, 512]),
                )
                nc.vector.tensor_tensor(att_sb[:, j, :], blk[:], rb[:], ALU.mult)

            for tt in range(4):
                x_sb = workp.tile([128, H], F32, tag="xsb", bufs=2)
                for ft in range(2):
                    op = ps.tile([128, 512], F32, tag="mm1", bufs=4)
                    for j in range(NC):
                        nc.tensor.matmul(
                            op[:],
                            att_sb[:, j, 128 * tt : 128 * (tt + 1)],
                            wot_sb[:, j, 512 * ft : 512 * (ft + 1)],
                            start=(j == 0),
                            stop=(j == NC - 1),
                        )
                    res_t = workp.tile([128, 512], F32, tag="res", bufs=2)
                    nc.sync.dma_start(
                        res_t[:],
                        resi.ap()[128 * tt : 128 * (tt + 1), 512 * ft : 512 * (ft + 1)],
                    )
                    nc.vector.tensor_tensor(
                        x_sb[:, 512 * ft : 512 * (ft + 1)], op[:], res_t[:], ALU.add
                    )

                bnst = workp.tile([128, 2, 6], F32, tag="bnst", bufs=2)
                nc.vector.bn_stats(bnst[:, 0, :], x_sb[:, 0:512])
                nc.vector.bn_stats(bnst[:, 1, :], x_sb[:, 512:1024])
                stats = workp.tile([128, 2], F32, tag="stats", bufs=2)
                nc.vector.bn_aggr(stats[:], bnst[:])
                std = workp.tile([128, 1], F32, tag="std", bufs=2)
                nc.scalar.activation(std[:], stats[:, 1:2], AF.Sqrt, bias=eps_sb[:])
                rstd = workp.tile([128, 1], F32, tag="rstd", bufs=2)
                nc.vector.reciprocal(rstd[:], std[:])
                nmr = workp.tile([128, 1], F32, tag="nmr", bufs=2)
                nc.vector.tensor_scalar(
                    nmr[:], stats[:, 0:1], rstd[:], -1.0, ALU.mult, ALU.mult
                )
                xh = workp.tile([128, H], F32, tag="xh", bufs=2)
                nc.vector.tensor_scalar(
                    xh[:], x_sb[:], rstd[:], nmr[:], ALU.mult, ALU.add
                )
                yt = workp.tile([128, H], F32, tag="yt", bufs=2)
                nc.vector.tensor_tensor(yt[:], xh[:], lnw_sb[:], ALU.mult)
                nc.vector.tensor_tensor(yt[:], yt[:], lnb_sb[:], ALU.add)
                nc.sync.dma_start(y.ap()[128 * tt : 128 * (tt + 1), :], yt[:])

    nc.compile()
    return nc


class _Runner:
    """Compiles the Bass program once and keeps a reusable sharded jit."""

    def __init__(self, build_fn=None):
        self.nc = (build_fn or _build_program)()
        self._sharded = None
        self._meta = None

    def _make_sharded(self):
        import jax
        from jax.sharding import Mesh, PartitionSpec
        from jax.experimental.shard_map import shard_map
        from concourse.bass2jax import (
            _bass_exec_p,
            install_neuronx_cc_hook,
            partition_id_tensor,
        )

        install_neuronx_cc_hook()
        nc = self.nc
        partition_name = (
            nc.partition_id_tensor.name if nc.partition_id_tensor else None
        )

        in_names, out_names, out_avals, zero_outs = [], [], [], []
        for alloc in nc.m.functions[0].allocations:
            if not isinstance(alloc, mybir.MemoryLocationSet):
                continue
            name = alloc.memorylocations[0].name
            if alloc.kind == "ExternalInput":
                if name != partition_name:
                    in_names.append(name)
            elif alloc.kind == "ExternalOutput":
                shape = tuple(alloc.tensor_shape)
                dtype = mybir.dt.np(alloc.dtype)
                out_names.append(name)
                out_avals.append(jax.core.ShapedArray(shape, dtype))
                zero_outs.append(np.zeros(shape, dtype))
        n_params = len(in_names)
        all_names = list(in_names) + list(out_names)
        if partition_name is not None:
            all_names.append(partition_name)

        def _body(*args):
            operands = list(args)
            if partition_name is not None:
                operands.append(partition_id_tensor())
            outs = _bass_exec_p.bind(
                *operands,
                out_avals=tuple(out_avals),
                in_names=tuple(all_names),
                out_names=tuple(out_names),
                lowering_input_output_aliases=(),
                sim_require_finite=True,
                sim_require_nnan=True,
                nc=nc,
            )
            return tuple(outs)

        devices = jax.devices()[:NC]
        mesh = Mesh(np.asarray(devices), ("core",))
        self._mesh = mesh
        n_outs = len(out_names)
        in_specs = (PartitionSpec("core"),) * (n_params + n_outs)
        out_specs = (PartitionSpec("core"),) * n_outs
        donate = tuple(range(n_params, n_params + n_outs))
        sharded = jax.jit(
            shard_map(
                _body, mesh=mesh, in_specs=in_specs, out_specs=out_specs, check_rep=False
            ),
            donate_argnums=donate,
            keep_unused=True,
        )
        self._meta = (in_names, out_names, out_avals, zero_outs)
        self._sharded = sharded

    def stage_inputs(self, in_maps):
        """device_put the concatenated inputs once; returns (ins_dev, zeros_dev)."""
        import jax
        from jax.sharding import NamedSharding, PartitionSpec

        if self._sharded is None:
            self._make_sharded()
        in_names, out_names, out_avals, zero_outs = self._meta
        sh = NamedSharding(self._mesh, PartitionSpec("core"))
        concat_in = [
            np.concatenate([np.asarray(m[name]) for m in in_maps], axis=0)
            for name in in_names
        ]
        concat_zeros = [
            np.zeros((NC * z.shape[0], *z.shape[1:]), z.dtype) for z in zero_outs
        ]
        ins_dev = [jax.device_put(a, sh) for a in concat_in]
        zeros_dev = [jax.device_put(a, sh) for a in concat_zeros]
        return ins_dev, zeros_dev

    def bench(self, in_maps, iters=20):
        """Steady-state seconds/call with device-resident inputs.

        Outputs are fully overwritten by the kernel, so each call's outputs are
        donated as the next call's output buffers (no H2D in the loop).
        """
        import jax
        import time

        ins_dev, zeros_dev = self.stage_inputs(in_maps)
        outs = self._sharded(*ins_dev, *zeros_dev)
        jax.block_until_ready(outs)
        t0 = time.time()
        for _ in range(iters):
            outs = self._sharded(*ins_dev, *outs)
        jax.block_until_ready(outs)
        return (time.time() - t0) / iters

    def run(self, in_maps):
        if self._sharded is None:
            self._make_sharded()
        in_names, out_names, out_avals, zero_outs = self._meta
        n_params = len(in_names)
        concat_in = [
            np.concatenate([np.asarray(m[name]) for m in in_maps], axis=0)
            for name in in_names
        ]
        concat_zeros = [
            np.zeros((NC * z.shape[0], *z.shape[1:]), z.dtype) for z in zero_outs
        ]
        out_arrs = self._sharded(*concat_in, *concat_zeros)
        return [
            {
                name: np.asarray(out_arrs[i]).reshape(NC, *out_avals[i].shape)[c]
                for i, name in enumerate(out_names)
            }
            for c in range(NC)
        ]


def _get_runner():
    global _RUNNER
    if _RUNNER is None:
        _RUNNER = _Runner()
    return _RUNNER


def _prep_in_maps(pre_out, att_mask, Wq, bq, Wk, bk, Wv, bv, Wo, bo, ln_w, ln_b):
    f32 = np.float32
    bf16 = ml_dtypes.bfloat16
    x = np.asarray(pre_out, f32).reshape(T, H)
    xT = np.ascontiguousarray(x.T).astype(bf16)

    m = (1.0 - np.asarray(att_mask, f32).reshape(B, S)) * -10000.0
    # column (b*KT + kt) holds mask for k-tokens [kt*128, (kt+1)*128) of batch b
    mneg = np.ascontiguousarray(m.reshape(B, KT, 128).transpose(2, 0, 1).reshape(128, B * KT))

    wot = np.ascontiguousarray(np.asarray(Wo, f32).T).astype(bf16)
    res_full = x + np.asarray(bo, f32)[None, :]
    lnw_b = np.ascontiguousarray(np.broadcast_to(np.asarray(ln_w, f32), (128, H)))
    lnb_b = np.ascontiguousarray(np.broadcast_to(np.asarray(ln_b, f32), (128, H)))

    Wq_, Wk_, Wv_ = (np.asarray(w, f32) for w in (Wq, Wk, Wv))
    bq_, bk_, bv_ = (np.asarray(v, f32) for v in (bq, bk, bv))

    in_maps = []
    for c in range(NC):
        fs = slice(128 * c, 128 * (c + 1))
        in_maps.append(
            {
                "xT": xT,
                "wq": np.ascontiguousarray(Wq_[fs].T).astype(bf16),
                "wk": np.ascontiguousarray(Wk_[fs].T).astype(bf16),
                "wv": np.ascontiguousarray(Wv_[fs].T).astype(bf16),
                "bq": np.ascontiguousarray(bq_[fs].reshape(128, 1)),
                "bk": np.ascontiguousarray(bk_[fs].reshape(128, 1)),
                "bv": np.ascontiguousarray(bv_[fs].reshape(1, 128)).astype(bf16),
                "mneg": mneg,
                "wot": wot,
                "resi": np.ascontiguousarray(res_full[TPC * c : TPC * (c + 1)]),
                "lnw": lnw_b,
                "lnb": lnb_b,
            }
        )
    return in_maps


def kernel(**inputs):
    runner = _get_runner()
    in_maps = _prep_in_maps(**inputs)
    results = runner.run(in_maps)
    y = np.concatenate([results[c]["y"] for c in range(NC)], axis=0)
    return y.reshape(B, S, H).astype(np.float32)


# revision 11
# speedup vs baseline: 7818.7616x; 6.4172x over previous
"""Trainium2 Bass kernel for a dense MHA layer (B=2, S=2048, H=1024, 16 heads)
with residual + LayerNorm, tensor-parallel over heads across 8 NeuronCores.

Per-core plan (core c owns heads 2c, 2c+1):
  phase 1: QKV projections from a shared transposed activation (xT), keeping
           Q^T/K^T feature-major and V token-major (with a ones column so the
           attention matmul also produces softmax denominators).
  phase 2: per (batch, head, q-stripe): scores^T = K Q^T on PE, exp via ACT
           (mask folded into the per-partition bias, 1/sqrt(hd) into the
           scale), then att^T = [V|1]^T E accumulated over k-tiles.
  phase 3: AllToAll re-shards from head-parallel to sequence-parallel
           (each core ends with all 1024 att features for its 512 tokens,
           plus the 16 per-head denominators as extra rows).
  phase 4: normalize, output projection, residual add, LayerNorm.

All matmuls run in bf16 with fp32 PSUM accumulation; softmax denominators and
the LayerNorm path stay fp32.
"""

import sys

for _p in ("/opt/trn_rl_repo", "/root/.axon_site/_ro/trn_rl_repo"):
    if _p not in sys.path:
        sys.path.append(_p)

import numpy as np
import ml_dtypes

import concourse.bacc as bacc
import concourse.tile as tile
import concourse.mybir as mybir
from concourse.bass_utils import run_bass_kernel_spmd

F32 = mybir.dt.float32
BF16 = mybir.dt.bfloat16
AF = mybir.ActivationFunctionType
ALU = mybir.AluOpType

NC = 8          # cores
H = 1024        # model dim
NH = 16         # heads
HD = 64         # head dim
B = 2
S = 2048
T = B * S       # 4096 tokens
TPC = T // NC   # 512 tokens per core (phase 4)
NSTR = T // 512  # 8 token stripes of 512
KT = S // 128   # 16 k-tiles per batch
QS = S // 512   # 4 q-stripes per batch
EPS = 1e-12

_RUNNER = None


def _build_program(passes=1):
    nc = bacc.Bacc("TRN2", target_bir_lowering=False, debug=False, num_devices=NC)

    xT = nc.dram_tensor("xT", [H, T], BF16, kind="ExternalInput")
    wq = nc.dram_tensor("wq", [H, 128], BF16, kind="ExternalInput")
    wk = nc.dram_tensor("wk", [H, 128], BF16, kind="ExternalInput")
    wv = nc.dram_tensor("wv", [H, 128], BF16, kind="ExternalInput")
    bq = nc.dram_tensor("bq", [128, 1], F32, kind="ExternalInput")
    bk = nc.dram_tensor("bk", [128, 1], F32, kind="ExternalInput")
    bv = nc.dram_tensor("bv", [1, 128], BF16, kind="ExternalInput")
    mneg = nc.dram_tensor("mneg", [128, B * KT], F32, kind="ExternalInput")
    wot = nc.dram_tensor("wot", [H, H], BF16, kind="ExternalInput")
    resi = nc.dram_tensor("resi", [TPC, H], F32, kind="ExternalInput")
    lnw = nc.dram_tensor("lnw", [128, H], F32, kind="ExternalInput")
    lnb = nc.dram_tensor("lnb", [128, H], F32, kind="ExternalInput")
    y = nc.dram_tensor("y", [TPC, H], F32, kind="ExternalOutput")

    with tile.TileContext(nc) as tc:
        with (
            tc.tile_pool(name="const", bufs=1) as constp,
            tc.tile_pool(name="pers", bufs=1) as pers,
            tc.tile_pool(name="work", bufs=2) as workp,
            tc.tile_pool(name="ps", bufs=1, space="PSUM") as ps,
            tc.tile_pool(name="dram", bufs=1, space="DRAM") as dram,
        ):
            # ---- constants / weights
            wq_sb = constp.tile([128, 8, 128], BF16)
            nc.sync.dma_start(wq_sb[:], wq.ap().rearrange("(k p) m -> p k m", p=128))
            wk_sb = constp.tile([128, 8, 128], BF16)
            nc.sync.dma_start(wk_sb[:], wk.ap().rearrange("(k p) m -> p k m", p=128))
            wv_sb = constp.tile([128, 8, 128], BF16)
            nc.sync.dma_start(wv_sb[:], wv.ap().rearrange("(k p) m -> p k m", p=128))
            wot_sb = constp.tile([128, 8, H], BF16)
            nc.sync.dma_start(wot_sb[:], wot.ap().rearrange("(j p) f -> p j f", p=128))
            bq_sb = constp.tile([128, 1], F32)
            nc.sync.dma_start(bq_sb[:], bq.ap())
            bk_sb = constp.tile([128, 1], F32)
            nc.sync.dma_start(bk_sb[:], bk.ap())
            bv_sb = constp.tile([1, 128], BF16)
            nc.sync.dma_start(bv_sb[:], bv.ap())
            mneg_sb = constp.tile([128, B * KT], F32)
            nc.sync.dma_start(mneg_sb[:], mneg.ap())
            lnw_sb = constp.tile([128, H], F32)
            nc.sync.dma_start(lnw_sb[:], lnw.ap())
            lnb_sb = constp.tile([128, H], F32)
            nc.sync.dma_start(lnb_sb[:], lnb.ap())
            ones_sb = constp.tile([1, 128], BF16)
            nc.vector.memset(ones_sb[:], 1.0)
            eps_sb = constp.tile([128, 1], F32)
            nc.vector.memset(eps_sb[:], EPS)

            qt_sb = pers.tile([128, T], BF16)   # Q^T (2 heads stacked)
            kt_sb = pers.tile([128, T], BF16)   # K^T
            v_sb = pers.tile([128, 32, 130], BF16)  # V tok-major + ones cols
            att_sb = pers.tile([128, 8, 512], BF16)  # normalized att^T (phase 4)
            nc.vector.memset(v_sb[:, :, 64:65], 1.0)
            nc.vector.memset(v_sb[:, :, 129:130], 1.0)

            a2a_in = dram.tile([NC, 130, 512], F32)
            a2a_out = dram.tile([NC, 130, 512], F32)

            xTr = xT.ap().rearrange("(k p) (s t) -> s p k t", p=128, t=512)

            for _pass in range(passes):
                _emit_body(
                    nc, tc, workp, ps,
                    wq_sb, wk_sb, wv_sb, wot_sb, bq_sb, bk_sb, bv_sb, mneg_sb,
                    lnw_sb, lnb_sb, ones_sb, eps_sb, qt_sb, kt_sb, v_sb, att_sb,
                    a2a_in, a2a_out, xTr, resi, y,
                )

    nc.compile()
    return nc


def _emit_body(
    nc, tc, workp, ps,
    wq_sb, wk_sb, wv_sb, wot_sb, bq_sb, bk_sb, bv_sb, mneg_sb,
    lnw_sb, lnb_sb, ones_sb, eps_sb, qt_sb, kt_sb, v_sb, att_sb,
    a2a_in, a2a_out, xTr, resi, y,
):
    if True:  # keep original indentation below
        if True:
            # ---- phase 1: QKV projections
            for s in range(NSTR):
                xs = workp.tile([128, 8, 512], BF16, tag="xs", bufs=2)
                nc.sync.dma_start(xs[:], xTr[s])

                qp = ps.tile([128, 512], F32, tag="mm1", bufs=4)
                for k in range(8):
                    nc.tensor.matmul(
                        qp[:], wq_sb[:, k, :], xs[:, k, :], start=(k == 0), stop=(k == 7)
                    )
                nc.scalar.activation(
                    qt_sb[:, 512 * s : 512 * (s + 1)], qp[:], AF.Identity, bias=bq_sb[:]
                )

                kp = ps.tile([128, 512], F32, tag="mm1", bufs=4)
                for k in range(8):
                    nc.tensor.matmul(
                        kp[:], wk_sb[:, k, :], xs[:, k, :], start=(k == 0), stop=(k == 7)
                    )
                nc.scalar.activation(
                    kt_sb[:, 512 * s : 512 * (s + 1)], kp[:], AF.Identity, bias=bk_sb[:]
                )

                for tt in range(4):
                    vp = ps.tile([128, 128], F32, tag="acc", bufs=2)
                    for k in range(8):
                        nc.tensor.matmul(
                            vp[:],
                            xs[:, k, 128 * tt : 128 * (tt + 1)],
                            wv_sb[:, k, :],
                            start=(k == 0),
                            stop=False,
                        )
                    nc.tensor.matmul(vp[:], ones_sb[:], bv_sb[:], start=False, stop=True)
                    g = s * 4 + tt
                    nc.vector.tensor_copy(v_sb[:, g, 0:64], vp[:, 0:64])
                    nc.vector.tensor_copy(v_sb[:, g, 65:129], vp[:, 64:128])

            # ---- phase 2: attention (per batch, local head, q-stripe)
            for b in range(B):
                for lh in range(2):
                    hr = 64 * lh
                    for qs in range(QS):
                        j = b * QS + qs
                        qcol = b * S + 512 * qs
                        e_sb = workp.tile([128, KT, 512], BF16, tag="e", bufs=2)
                        for kt in range(KT):
                            kcol = b * S + 128 * kt
                            sp = ps.tile([128, 512], F32, tag="mm1", bufs=4)
                            nc.tensor.matmul(
                                sp[:],
                                kt_sb[hr : hr + 64, kcol : kcol + 128],
                                qt_sb[hr : hr + 64, qcol : qcol + 512],
                                start=True,
                                stop=True,
                            )
                            nc.scalar.activation(
                                e_sb[:, kt, :],
                                sp[:],
                                AF.Exp,
                                bias=mneg_sb[:, b * KT + kt : b * KT + kt + 1],
                                scale=1.0 / np.sqrt(HD),
                            )
                        av = ps.tile([65, 512], F32, tag="acc", bufs=2)
                        for kt in range(KT):
                            g = b * KT + kt
                            nc.tensor.matmul(
                                av[:],
                                v_sb[:, g, 65 * lh : 65 * lh + 65],
                                e_sb[:, kt, :],
                                start=(kt == 0),
                                stop=(kt == KT - 1),
                            )
                        avs = workp.tile([65, 512], F32, tag="avs", bufs=2)
                        nc.vector.tensor_copy(avs[:], av[:])
                        nc.sync.dma_start(a2a_in[j, 64 * lh : 64 * lh + 64, :], avs[0:64, :])
                        nc.sync.dma_start(a2a_in[j, 128 + lh : 129 + lh, :], avs[64:65, :])

            # ---- phase 3: AllToAll (head-parallel -> sequence-parallel)
            nc.gpsimd.collective_compute(
                "AllToAll",
                ALU.bypass,
                replica_groups=[list(range(NC))],
                ins=[a2a_in.opt()],
                outs=[a2a_out.opt()],
            )

            # ---- phase 4: normalize + output projection + residual + LayerNorm
            sums_sb = workp.tile([16, 512], F32, tag="sums", bufs=1)
            nc.sync.dma_start(sums_sb[:], a2a_out[0:NC, 128:130, :])
            recip_sb = workp.tile([16, 512], F32, tag="recip", bufs=1)
            nc.vector.reciprocal(recip_sb[:], sums_sb[:])

            for j in range(NC):
                blk = workp.tile([128, 512], F32, tag="blk", bufs=2)
                nc.sync.dma_start(blk[:], a2a_out[j, 0:128, :])
                rb = workp.tile([128, 512], F32, tag="rb", bufs=2)
                nc.sync.dma_start(
                    rb[:],
                    recip_sb[2 * j : 2 * j + 2, :].unsqueeze(1).broadcast_to([2, 64, 512]),
                )
                nc.vector.tensor_tensor(att_sb[:, j, :], blk[:], rb[:], ALU.mult)

            for tt in range(4):
                x_sb = workp.tile([128, H], F32, tag="xsb", bufs=2)
                for ft in range(2):
                    op = ps.tile([128, 512], F32, tag="mm1", bufs=4)
                    for j in range(NC):
                        nc.tensor.matmul(
                            op[:],
                            att_sb[:, j, 128 * tt : 128 * (tt + 1)],
                            wot_sb[:, j, 512 * ft : 512 * (ft + 1)],
                            start=(j == 0),
                            stop=(j == NC - 1),
                        )
                    res_t = workp.tile([128, 512], F32, tag="res", bufs=2)
                    nc.sync.dma_start(
                        res_t[:],
                        resi.ap()[128 * tt : 128 * (tt + 1), 512 * ft : 512 * (ft + 1)],
                    )
                    nc.vector.tensor_tensor(
                        x_sb[:, 512 * ft : 512 * (ft + 1)], op[:], res_t[:], ALU.add
                    )

                bnst = workp.tile([128, 2, 6], F32, tag="bnst", bufs=2)
                nc.vector.bn_stats(bnst[:, 0, :], x_sb[:, 0:512])
                nc.vector.bn_stats(bnst[:, 1, :], x_sb[:, 512:1024])
                stats = workp.tile([128, 2], F32, tag="stats", bufs=2)
                nc.vector.bn_aggr(stats[:], bnst[:])
                std = workp.tile([128, 1], F32, tag="std", bufs=2)
                nc.scalar.activation(std[:], stats[:, 1:2], AF.Sqrt, bias=eps_sb[:])
                rstd = workp.tile([128, 1], F32, tag="rstd", bufs=2)
                nc.vector.reciprocal(rstd[:], std[:])
                nmr = workp.tile([128, 1], F32, tag="nmr", bufs=2)
                nc.vector.tensor_scalar(
                    nmr[:], stats[:, 0:1], rstd[:], -1.0, ALU.mult, ALU.mult
                )
                xh = workp.tile([128, H], F32, tag="xh", bufs=2)
                nc.vector.tensor_scalar(
                    xh[:], x_sb[:], rstd[:], nmr[:], ALU.mult, ALU.add
                )
                yt = workp.tile([128, H], F32, tag="yt", bufs=2)
                nc.vector.tensor_tensor(yt[:], xh[:], lnw_sb[:], ALU.mult)
                nc.vector.tensor_tensor(yt[:], yt[:], lnb_sb[:], ALU.add)
                nc.sync.dma_start(y.ap()[128 * tt : 128 * (tt + 1), :], yt[:])


class _Runner:
    """Compiles the Bass program once and keeps a reusable sharded jit."""

    def __init__(self, build_fn=None):
        self.nc = (build_fn or _build_program)()
        self._sharded = None
        self._meta = None

    def _make_sharded(self):
        import jax
        from jax.sharding import Mesh, PartitionSpec
        from jax.experimental.shard_map import shard_map
        from concourse.bass2jax import (
            _bass_exec_p,
            install_neuronx_cc_hook,
            partition_id_tensor,
        )

        install_neuronx_cc_hook()
        nc = self.nc
        partition_name = (
            nc.partition_id_tensor.name if nc.partition_id_tensor else None
        )

        in_names, out_names, out_avals, zero_outs = [], [], [], []
        for alloc in nc.m.functions[0].allocations:
            if not isinstance(alloc, mybir.MemoryLocationSet):
                continue
            name = alloc.memorylocations[0].name
            if alloc.kind == "ExternalInput":
                if name != partition_name:
                    in_names.append(name)
            elif alloc.kind == "ExternalOutput":
                shape = tuple(alloc.tensor_shape)
                dtype = mybir.dt.np(alloc.dtype)
                out_names.append(name)
                out_avals.append(jax.core.ShapedArray(shape, dtype))
                zero_outs.append(np.zeros(shape, dtype))
        n_params = len(in_names)
        all_names = list(in_names) + list(out_names)
        if partition_name is not None:
            all_names.append(partition_name)

        def _body(*args):
            operands = list(args)
            if partition_name is not None:
                operands.append(partition_id_tensor())
            outs = _bass_exec_p.bind(
                *operands,
                out_avals=tuple(out_avals),
                in_names=tuple(all_names),
                out_names=tuple(out_names),
                lowering_input_output_aliases=(),
                sim_require_finite=True,
                sim_require_nnan=True,
                nc=nc,
            )
            return tuple(outs)

        devices = jax.devices()[:NC]
        mesh = Mesh(np.asarray(devices), ("core",))
        self._mesh = mesh
        n_outs = len(out_names)
        in_specs = (PartitionSpec("core"),) * (n_params + n_outs)
        out_specs = (PartitionSpec("core"),) * n_outs
        donate = tuple(range(n_params, n_params + n_outs))
        sharded = jax.jit(
            shard_map(
                _body, mesh=mesh, in_specs=in_specs, out_specs=out_specs, check_rep=False
            ),
            donate_argnums=donate,
            keep_unused=True,
        )
        self._meta = (in_names, out_names, out_avals, zero_outs)
        self._sharded = sharded

    def stage_inputs(self, in_maps):
        """device_put the concatenated inputs once; returns (ins_dev, zeros_dev)."""
        import jax
        from jax.sharding import NamedSharding, PartitionSpec

        if self._sharded is None:
            self._make_sharded()
        in_names, out_names, out_avals, zero_outs = self._meta
        sh = NamedSharding(self._mesh, PartitionSpec("core"))
        concat_in = [
            np.concatenate([np.asarray(m[name]) for m in in_maps], axis=0)
            for name in in_names
        ]
        concat_zeros = [
            np.zeros((NC * z.shape[0], *z.shape[1:]), z.dtype) for z in zero_outs
        ]
        ins_dev = [jax.device_put(a, sh) for a in concat_in]
        zeros_dev = [jax.device_put(a, sh) for a in concat_zeros]
        return ins_dev, zeros_dev

    def bench(self, in_maps, iters=20):
        """Steady-state seconds/call with device-resident inputs.

        Outputs are fully overwritten by the kernel, so each call's outputs are
        donated as the next call's output buffers (no H2D in the loop).
        """
        import jax
        import time

        ins_dev, zeros_dev = self.stage_inputs(in_maps)
        outs = self._sharded(*ins_dev, *zeros_dev)
        jax.block_until_ready(outs)
        t0 = time.time()
        for _ in range(iters):
            outs = self._sharded(*ins_dev, *outs)
        jax.block_until_ready(outs)
        return (time.time() - t0) / iters

    def run(self, in_maps):
        if self._sharded is None:
            self._make_sharded()
        in_names, out_names, out_avals, zero_outs = self._meta
        n_params = len(in_names)
        concat_in = [
            np.concatenate([np.asarray(m[name]) for m in in_maps], axis=0)
            for name in in_names
        ]
        concat_zeros = [
            np.zeros((NC * z.shape[0], *z.shape[1:]), z.dtype) for z in zero_outs
        ]
        out_arrs = self._sharded(*concat_in, *concat_zeros)
        return [
            {
                name: np.asarray(out_arrs[i]).reshape(NC, *out_avals[i].shape)[c]
                for i, name in enumerate(out_names)
            }
            for c in range(NC)
        ]


def _get_runner():
    global _RUNNER
    if _RUNNER is None:
        _RUNNER = _Runner()
    return _RUNNER


def _prep_in_maps(pre_out, att_mask, Wq, bq, Wk, bk, Wv, bv, Wo, bo, ln_w, ln_b):
    f32 = np.float32
    bf16 = ml_dtypes.bfloat16
    x = np.asarray(pre_out, f32).reshape(T, H)
    xT = np.ascontiguousarray(x.T).astype(bf16)

    m = (1.0 - np.asarray(att_mask, f32).reshape(B, S)) * -10000.0
    # column (b*KT + kt) holds mask for k-tokens [kt*128, (kt+1)*128) of batch b
    mneg = np.ascontiguousarray(m.reshape(B, KT, 128).transpose(2, 0, 1).reshape(128, B * KT))

    wot = np.ascontiguousarray(np.asarray(Wo, f32).T).astype(bf16)
    res_full = x + np.asarray(bo, f32)[None, :]
    lnw_b = np.ascontiguousarray(np.broadcast_to(np.asarray(ln_w, f32), (128, H)))
    lnb_b = np.ascontiguousarray(np.broadcast_to(np.asarray(ln_b, f32), (128, H)))

    Wq_, Wk_, Wv_ = (np.asarray(w, f32) for w in (Wq, Wk, Wv))
    bq_, bk_, bv_ = (np.asarray(v, f32) for v in (bq, bk, bv))

    in_maps = []
    for c in range(NC):
        fs = slice(128 * c, 128 * (c + 1))
        in_maps.append(
            {
                "xT": xT,
                "wq": np.ascontiguousarray(Wq_[fs].T).astype(bf16),
                "wk": np.ascontiguousarray(Wk_[fs].T).astype(bf16),
                "wv": np.ascontiguousarray(Wv_[fs].T).astype(bf16),
                "bq": np.ascontiguousarray(bq_[fs].reshape(128, 1)),
                "bk": np.ascontiguousarray(bk_[fs].reshape(128, 1)),
                "bv": np.ascontiguousarray(bv_[fs].reshape(1, 128)).astype(bf16),
                "mneg": mneg,
                "wot": wot,
                "resi": np.ascontiguousarray(res_full[TPC * c : TPC * (c + 1)]),
                "lnw": lnw_b,
                "lnb": lnb_b,
            }
        )
    return in_maps


def kernel(**inputs):
    runner = _get_runner()
    in_maps = _prep_in_maps(**inputs)
    results = runner.run(in_maps)
    y = np.concatenate([results[c]["y"] for c in range(NC)], axis=0)
    return y.reshape(B, S, H).astype(np.float32)


# revision 23
# speedup vs baseline: 13607.2596x; 1.7403x over previous
"""Trainium2 Bass kernel for a dense MHA layer (B=2, S=2048, H=1024, 16 heads)
with residual + LayerNorm, tensor-parallel over heads across 8 NeuronCores.

Per-core plan (core c owns heads 2c, 2c+1):
  phase 1: QKV projections from a shared transposed activation (xT), keeping
           Q^T/K^T feature-major and V token-major (with a ones column so the
           attention matmul also produces softmax denominators).
  phase 2: per (batch, head, q-stripe): scores^T = K Q^T on PE, exp via ACT
           (mask folded into the per-partition bias, 1/sqrt(hd) into the
           scale), then att^T = [V|1]^T E accumulated over k-tiles.
  phase 3: AllToAll re-shards from head-parallel to sequence-parallel
           (each core ends with all 1024 att features for its 512 tokens,
           plus the 16 per-head denominators as extra rows).
  phase 4: normalize, output projection, residual add, LayerNorm.

All matmuls run in bf16 with fp32 PSUM accumulation; softmax denominators and
the LayerNorm path stay fp32.
"""

import sys

for _p in ("/opt/trn_rl_repo", "/root/.axon_site/_ro/trn_rl_repo"):
    if _p not in sys.path:
        sys.path.append(_p)

import numpy as np
import ml_dtypes

import concourse.bacc as bacc
import concourse.tile as tile
import concourse.mybir as mybir
from concourse.bass_utils import run_bass_kernel_spmd

F32 = mybir.dt.float32
BF16 = mybir.dt.bfloat16
AF = mybir.ActivationFunctionType
ALU = mybir.AluOpType

NC = 8          # cores
H = 1024        # model dim
NH = 16         # heads
HD = 64         # head dim
B = 2
S = 2048
T = B * S       # 4096 tokens
TPC = T // NC   # 512 tokens per core (phase 4)
NSTR = T // 512  # 8 token stripes of 512
KT = S // 128   # 16 k-tiles per batch
QS = S // 512   # 4 q-stripes per batch
EPS = 1e-12

_RUNNER = None


def _build_program(passes=1, single_core=False):
    nc = bacc.Bacc(
        "TRN2",
        target_bir_lowering=False,
        debug=False,
        num_devices=1 if single_core else NC,
    )

    xT = nc.dram_tensor("xT", [H, T], BF16, kind="ExternalInput")
    wq = nc.dram_tensor("wq", [H, 128], BF16, kind="ExternalInput")
    wk = nc.dram_tensor("wk", [H, 128], BF16, kind="ExternalInput")
    wv = nc.dram_tensor("wv", [H, 128], BF16, kind="ExternalInput")
    bq = nc.dram_tensor("bq", [128, 1], F32, kind="ExternalInput")
    bk = nc.dram_tensor("bk", [128, 1], F32, kind="ExternalInput")
    bv = nc.dram_tensor("bv", [1, 128], BF16, kind="ExternalInput")
    mneg = nc.dram_tensor("mneg", [128, B * KT], F32, kind="ExternalInput")
    wot = nc.dram_tensor("wot", [H, H], BF16, kind="ExternalInput")
    resi = nc.dram_tensor("resi", [TPC, H], F32, kind="ExternalInput")
    lnw = nc.dram_tensor("lnw", [128, H], F32, kind="ExternalInput")
    lnb = nc.dram_tensor("lnb", [128, H], F32, kind="ExternalInput")
    y = nc.dram_tensor("y", [TPC, H], F32, kind="ExternalOutput")

    with tile.TileContext(nc) as tc:
        with (
            tc.tile_pool(name="const", bufs=1) as constp,
            tc.tile_pool(name="pers", bufs=1) as pers,
            tc.tile_pool(name="work", bufs=2) as workp,
            tc.tile_pool(name="ps", bufs=1, space="PSUM") as ps,
            tc.tile_pool(name="dram", bufs=1, space="DRAM") as dram,
        ):
            # ---- constants / weights
            wq_sb = constp.tile([128, 8, 128], BF16)
            nc.sync.dma_start(wq_sb[:], wq.ap().rearrange("(k p) m -> p k m", p=128))
            wk_sb = constp.tile([128, 8, 128], BF16)
            nc.sync.dma_start(wk_sb[:], wk.ap().rearrange("(k p) m -> p k m", p=128))
            wv_sb = constp.tile([128, 8, 128], BF16)
            nc.sync.dma_start(wv_sb[:], wv.ap().rearrange("(k p) m -> p k m", p=128))
            wot_sb = constp.tile([128, 8, H], BF16)
            bq_sb = constp.tile([128, 1], F32)
            nc.sync.dma_start(bq_sb[:], bq.ap())
            bk_sb = constp.tile([128, 1], F32)
            nc.sync.dma_start(bk_sb[:], bk.ap())
            bv_sb = constp.tile([1, 128], BF16)
            nc.sync.dma_start(bv_sb[:], bv.ap())
            mneg_sb = constp.tile([128, B * KT], F32)
            nc.sync.dma_start(mneg_sb[:], mneg.ap())
            lnw_sb = constp.tile([128, H], F32)
            nc.sync.dma_start(lnw_sb[:], lnw.ap())
            lnb_sb = constp.tile([128, H], F32)
            nc.sync.dma_start(lnb_sb[:], lnb.ap())
            ones_sb = constp.tile([1, 128], BF16)
            nc.vector.memset(ones_sb[:], 1.0)
            eps_sb = constp.tile([128, 1], F32)
            nc.vector.memset(eps_sb[:], EPS)

            qt_sb = pers.tile([128, T], BF16)   # Q^T (2 heads stacked)
            kt_sb = pers.tile([128, T], BF16)   # K^T
            v_sb = pers.tile([128, 32, 130], BF16)  # V tok-major + ones cols
            att_sb = pers.tile([128, 8, 512], BF16)  # normalized att^T (phase 4)
            nc.vector.memset(v_sb[:, :, 64:65], 1.0)
            nc.vector.memset(v_sb[:, :, 129:130], 1.0)

            a2a_in = dram.tile([NC, 130, 512], F32)
            a2a_out = dram.tile([NC, 130, 512], F32)

            xTr = xT.ap().rearrange("(k p) (s t) -> s p k t", p=128, t=512)

            for _pass in range(passes):
                _emit_body(
                    nc, tc, workp, ps,
                    wq_sb, wk_sb, wv_sb, wot_sb, bq_sb, bk_sb, bv_sb, mneg_sb,
                    lnw_sb, lnb_sb, ones_sb, eps_sb, qt_sb, kt_sb, v_sb, att_sb,
                    a2a_in, a2a_out, xTr, resi, y, wot, single_core,
                )

    nc.compile()
    return nc


def _emit_body(
    nc, tc, workp, ps,
    wq_sb, wk_sb, wv_sb, wot_sb, bq_sb, bk_sb, bv_sb, mneg_sb,
    lnw_sb, lnb_sb, ones_sb, eps_sb, qt_sb, kt_sb, v_sb, att_sb,
    a2a_in, a2a_out, xTr, resi, y, wot=None, single_core=False,
):
    if True:  # keep original indentation below
        if True:
            # ---- phase 1: QKV projections
            for s in range(NSTR):
                xs = workp.tile([128, 8, 512], BF16, tag="xs", bufs=2)
                nc.sync.dma_start(xs[:], xTr[s])

                qp = ps.tile([128, 512], F32, tag="mm1", bufs=2)
                for k in range(8):
                    nc.tensor.matmul(
                        qp[:], wq_sb[:, k, :], xs[:, k, :], start=(k == 0), stop=(k == 7)
                    )
                nc.vector.tensor_scalar_add(
                    qt_sb[:, 512 * s : 512 * (s + 1)], qp[:], bq_sb[:]
                )

                kp = ps.tile([128, 512], F32, tag="mm1", bufs=2)
                for k in range(8):
                    nc.tensor.matmul(
                        kp[:], wk_sb[:, k, :], xs[:, k, :], start=(k == 0), stop=(k == 7)
                    )
                nc.vector.tensor_scalar_add(
                    kt_sb[:, 512 * s : 512 * (s + 1)], kp[:], bk_sb[:]
                )

                for tt in range(4):
                    vp = ps.tile([128, 128], F32, tag="acc", bufs=2)
                    for k in range(8):
                        nc.tensor.matmul(
                            vp[:],
                            xs[:, k, 128 * tt : 128 * (tt + 1)],
                            wv_sb[:, k, :],
                            start=(k == 0),
                            stop=False,
                        )
                    nc.tensor.matmul(vp[:], ones_sb[:], bv_sb[:], start=False, stop=True)
                    g = s * 4 + tt
                    nc.vector.tensor_copy(v_sb[:, g, 0:64], vp[:, 0:64])
                    nc.vector.tensor_copy(v_sb[:, g, 65:129], vp[:, 64:128])

            # ---- phase 2: attention (two q-stripes per exp tile to amortize
            # the ACT per-instruction PSUM-access bubble)
            for b in range(B):
                for lh in range(2):
                    hr = 64 * lh
                    for q2 in range(QS // 2):
                        qcol = b * S + 1024 * q2
                        e_sb = workp.tile([128, KT, 1024], BF16, tag="e", bufs=2)
                        for kt in range(KT):
                            kcol = b * S + 128 * kt
                            sp = ps.tile([128, 1024], F32, tag="mm1", bufs=2)
                            for half in range(2):
                                nc.tensor.matmul(
                                    sp[:, 512 * half : 512 * (half + 1)],
                                    kt_sb[hr : hr + 64, kcol : kcol + 128],
                                    qt_sb[
                                        hr : hr + 64,
                                        qcol + 512 * half : qcol + 512 * (half + 1),
                                    ],
                                    start=True,
                                    stop=True,
                                )
                            nc.scalar.activation(
                                e_sb[:, kt, :],
                                sp[:],
                                AF.Exp,
                                bias=mneg_sb[:, b * KT + kt : b * KT + kt + 1],
                                scale=1.0 / np.sqrt(HD),
                            )
                        av = ps.tile([65, 1024], F32, tag="acc", bufs=2)
                        for kt in range(KT):
                            g = b * KT + kt
                            for half in range(2):
                                nc.tensor.matmul(
                                    av[:, 512 * half : 512 * (half + 1)],
                                    v_sb[:, g, 65 * lh : 65 * lh + 65],
                                    e_sb[:, kt, 512 * half : 512 * (half + 1)],
                                    start=(kt == 0),
                                    stop=(kt == KT - 1),
                                )
                        avs = workp.tile([65, 1024], F32, tag="avs", bufs=2)
                        nc.vector.tensor_copy(avs[:], av[:])
                        for half in range(2):
                            j = b * QS + 2 * q2 + half
                            nc.sync.dma_start(
                                a2a_in[j, 64 * lh : 64 * lh + 64, :],
                                avs[0:64, 512 * half : 512 * (half + 1)],
                            )
                            nc.sync.dma_start(
                                a2a_in[j, 128 + lh : 129 + lh, :],
                                avs[64:65, 512 * half : 512 * (half + 1)],
                            )

            # ---- phase 3: AllToAll (head-parallel -> sequence-parallel)
            if single_core:
                # timing stand-in for TimelineSim (no collectives there)
                nc.sync.dma_start(a2a_out[:], a2a_in[:])
            else:
                nc.gpsimd.collective_compute(
                    "AllToAll",
                    ALU.bypass,
                    replica_groups=[list(range(NC))],
                    ins=[a2a_in.opt()],
                    outs=[a2a_out.opt()],
                )

            # ---- phase 4: normalize + output projection + residual + LayerNorm
            # Wo load deferred to here so startup DMA bandwidth goes to xT/QKV;
            # it overlaps the collective.
            nc.sync.dma_start(wot_sb[:], wot.ap().rearrange("(j p) f -> p j f", p=128))
            sums_sb = workp.tile([16, 512], F32, tag="sums", bufs=1)
            nc.sync.dma_start(sums_sb[:], a2a_out[0:NC, 128:130, :])
            recip_sb = workp.tile([16, 512], F32, tag="recip", bufs=1)
            nc.vector.reciprocal(recip_sb[:], sums_sb[:])

            for j in range(NC):
                blk = workp.tile([128, 512], F32, tag="blk", bufs=2)
                nc.sync.dma_start(blk[:], a2a_out[j, 0:128, :])
                rb = workp.tile([128, 512], F32, tag="rb", bufs=2)
                nc.sync.dma_start(
                    rb[:],
                    recip_sb[2 * j : 2 * j + 2, :].unsqueeze(1).broadcast_to([2, 64, 512]),
                )
                nc.vector.tensor_tensor(att_sb[:, j, :], blk[:], rb[:], ALU.mult)

            for tt in range(4):
                x_sb = workp.tile([128, H], F32, tag="xsb", bufs=2)
                for ft in range(2):
                    op = ps.tile([128, 512], F32, tag="mm1", bufs=2)
                    for j in range(NC):
                        nc.tensor.matmul(
                            op[:],
                            att_sb[:, j, 128 * tt : 128 * (tt + 1)],
                            wot_sb[:, j, 512 * ft : 512 * (ft + 1)],
                            start=(j == 0),
                            stop=(j == NC - 1),
                        )
                    res_t = workp.tile([128, 512], F32, tag="res", bufs=2)
                    nc.sync.dma_start(
                        res_t[:],
                        resi.ap()[128 * tt : 128 * (tt + 1), 512 * ft : 512 * (ft + 1)],
                    )
                    nc.vector.tensor_tensor(
                        x_sb[:, 512 * ft : 512 * (ft + 1)], op[:], res_t[:], ALU.add
                    )

                bnst = workp.tile([128, 2, 6], F32, tag="bnst", bufs=2)
                nc.vector.bn_stats(bnst[:, 0, :], x_sb[:, 0:512])
                nc.vector.bn_stats(bnst[:, 1, :], x_sb[:, 512:1024])
                stats = workp.tile([128, 2], F32, tag="stats", bufs=2)
                nc.vector.bn_aggr(stats[:], bnst[:])
                std = workp.tile([128, 1], F32, tag="std", bufs=2)
                nc.scalar.activation(std[:], stats[:, 1:2], AF.Sqrt, bias=eps_sb[:])
                rstd = workp.tile([128, 1], F32, tag="rstd", bufs=2)
                nc.vector.reciprocal(rstd[:], std[:])
                nmr = workp.tile([128, 1], F32, tag="nmr", bufs=2)
                nc.vector.tensor_scalar(
                    nmr[:], stats[:, 0:1], rstd[:], -1.0, ALU.mult, ALU.mult
                )
                xh = workp.tile([128, H], F32, tag="xh", bufs=2)
                # affine on ACT (idle in the tail); w/b on DVE, in place
                nc.scalar.activation(
                    xh[:], x_sb[:], AF.Identity, bias=nmr[:], scale=rstd[:]
                )
                nc.vector.tensor_tensor(xh[:], xh[:], lnw_sb[:], ALU.mult)
                nc.vector.tensor_tensor(xh[:], xh[:], lnb_sb[:], ALU.add)
                nc.sync.dma_start(y.ap()[128 * tt : 128 * (tt + 1), :], xh[:])


class _Runner:
    """Compiles the Bass program once and keeps a reusable sharded jit."""

    def __init__(self, build_fn=None):
        self.nc = (build_fn or _build_program)()
        self._sharded = None
        self._meta = None

    def _make_sharded(self):
        import jax
        from jax.sharding import Mesh, PartitionSpec
        from jax.experimental.shard_map import shard_map
        from concourse.bass2jax import (
            _bass_exec_p,
            install_neuronx_cc_hook,
            partition_id_tensor,
        )

        install_neuronx_cc_hook()
        nc = self.nc
        partition_name = (
            nc.partition_id_tensor.name if nc.partition_id_tensor else None
        )

        in_names, out_names, out_avals, zero_outs = [], [], [], []
        for alloc in nc.m.functions[0].allocations:
            if not isinstance(alloc, mybir.MemoryLocationSet):
                continue
            name = alloc.memorylocations[0].name
            if alloc.kind == "ExternalInput":
                if name != partition_name:
                    in_names.append(name)
            elif alloc.kind == "ExternalOutput":
                shape = tuple(alloc.tensor_shape)
                dtype = mybir.dt.np(alloc.dtype)
                out_names.append(name)
                out_avals.append(jax.core.ShapedArray(shape, dtype))
                zero_outs.append(np.zeros(shape, dtype))
        n_params = len(in_names)
        all_names = list(in_names) + list(out_names)
        if partition_name is not None:
            all_names.append(partition_name)

        def _body(*args):
            operands = list(args)
            if partition_name is not None:
                operands.append(partition_id_tensor())
            outs = _bass_exec_p.bind(
                *operands,
                out_avals=tuple(out_avals),
                in_names=tuple(all_names),
                out_names=tuple(out_names),
                lowering_input_output_aliases=(),
                sim_require_finite=True,
                sim_require_nnan=True,
                nc=nc,
            )
            return tuple(outs)

        devices = jax.devices()[:NC]
        mesh = Mesh(np.asarray(devices), ("core",))
        self._mesh = mesh
        n_outs = len(out_names)
        in_specs = (PartitionSpec("core"),) * (n_params + n_outs)
        out_specs = (PartitionSpec("core"),) * n_outs
        donate = tuple(range(n_params, n_params + n_outs))
        sharded = jax.jit(
            shard_map(
                _body, mesh=mesh, in_specs=in_specs, out_specs=out_specs, check_rep=False
            ),
            donate_argnums=donate,
            keep_unused=True,
        )
        self._meta = (in_names, out_names, out_avals, zero_outs)
        self._sharded = sharded

    def stage_inputs(self, in_maps):
        """device_put the concatenated inputs once; returns (ins_dev, zeros_dev)."""
        import jax
        from jax.sharding import NamedSharding, PartitionSpec

        if self._sharded is None:
            self._make_sharded()
        in_names, out_names, out_avals, zero_outs = self._meta
        sh = NamedSharding(self._mesh, PartitionSpec("core"))
        concat_in = [
            np.concatenate([np.asarray(m[name]) for m in in_maps], axis=0)
            for name in in_names
        ]
        concat_zeros = [
            np.zeros((NC * z.shape[0], *z.shape[1:]), z.dtype) for z in zero_outs
        ]
        ins_dev = [jax.device_put(a, sh) for a in concat_in]
        zeros_dev = [jax.device_put(a, sh) for a in concat_zeros]
        return ins_dev, zeros_dev

    def bench(self, in_maps, iters=20):
        """Steady-state seconds/call with device-resident inputs.

        Outputs are fully overwritten by the kernel, so each call's outputs are
        donated as the next call's output buffers (no H2D in the loop).
        """
        import jax
        import time

        ins_dev, zeros_dev = self.stage_inputs(in_maps)
        outs = self._sharded(*ins_dev, *zeros_dev)
        jax.block_until_ready(outs)
        t0 = time.time()
        for _ in range(iters):
            outs = self._sharded(*ins_dev, *outs)
        jax.block_until_ready(outs)
        return (time.time() - t0) / iters

    def run(self, in_maps):
        if self._sharded is None:
            self._make_sharded()
        in_names, out_names, out_avals, zero_outs = self._meta
        n_params = len(in_names)
        concat_in = [
            np.concatenate([np.asarray(m[name]) for m in in_maps], axis=0)
            for name in in_names
        ]
        concat_zeros = [
            np.zeros((NC * z.shape[0], *z.shape[1:]), z.dtype) for z in zero_outs
        ]
        out_arrs = self._sharded(*concat_in, *concat_zeros)
        return [
            {
                name: np.asarray(out_arrs[i]).reshape(NC, *out_avals[i].shape)[c]
                for i, name in enumerate(out_names)
            }
            for c in range(NC)
        ]


def _get_runner():
    global _RUNNER
    if _RUNNER is None:
        _RUNNER = _Runner()
    return _RUNNER


def _prep_in_maps(pre_out, att_mask, Wq, bq, Wk, bk, Wv, bv, Wo, bo, ln_w, ln_b):
    f32 = np.float32
    bf16 = ml_dtypes.bfloat16
    x = np.asarray(pre_out, f32).reshape(T, H)
    xT = np.ascontiguousarray(x.T).astype(bf16)

    m = (1.0 - np.asarray(att_mask, f32).reshape(B, S)) * -10000.0
    # column (b*KT + kt) holds mask for k-tokens [kt*128, (kt+1)*128) of batch b
    mneg = np.ascontiguousarray(m.reshape(B, KT, 128).transpose(2, 0, 1).reshape(128, B * KT))

    wot = np.ascontiguousarray(np.asarray(Wo, f32).T).astype(bf16)
    res_full = x + np.asarray(bo, f32)[None, :]
    lnw_b = np.ascontiguousarray(np.broadcast_to(np.asarray(ln_w, f32), (128, H)))
    lnb_b = np.ascontiguousarray(np.broadcast_to(np.asarray(ln_b, f32), (128, H)))

    Wq_, Wk_, Wv_ = (np.asarray(w, f32) for w in (Wq, Wk, Wv))
    bq_, bk_, bv_ = (np.asarray(v, f32) for v in (bq, bk, bv))

    in_maps = []
    for c in range(NC):
        fs = slice(128 * c, 128 * (c + 1))
        in_maps.append(
            {
                "xT": xT,
                "wq": np.ascontiguousarray(Wq_[fs].T).astype(bf16),
                "wk": np.ascontiguousarray(Wk_[fs].T).astype(bf16),
                "wv": np.ascontiguousarray(Wv_[fs].T).astype(bf16),
                "bq": np.ascontiguousarray(bq_[fs].reshape(128, 1)),
                "bk": np.ascontiguousarray(bk_[fs].reshape(128, 1)),
                "bv": np.ascontiguousarray(bv_[fs].reshape(1, 128)).astype(bf16),
                "mneg": mneg,
                "wot": wot,
                "resi": np.ascontiguousarray(res_full[TPC * c : TPC * (c + 1)]),
                "lnw": lnw_b,
                "lnb": lnb_b,
            }
        )
    return in_maps


def kernel(**inputs):
    runner = _get_runner()
    in_maps = _prep_in_maps(**inputs)
    results = runner.run(in_maps)
    y = np.concatenate([results[c]["y"] for c in range(NC)], axis=0)
    return y.reshape(B, S, H).astype(np.float32)
